# revision 1
# baseline (speedup 1.0000x reference)
"""
Trainium2 Bass kernel for DynamicGraphAttention
(softmax(Hn Wq^T (Hn Wk^T)^T / sqrt(D) + eta*logit(clip(A)) masked)).

Shapes (hardcoded):
  Hn     [16, 2048, 256] f32
  A_stat [2048, 2048]    f32
  M_mask [2048, 2048]    int32
  Wq, Wk [256, 256]      f32
  out    [16, 2048, 2048] f32

Sharding across 8 NeuronCores: 2 batch-groups x 4 seq(query)-groups.
Core c handles batches of group bg = c // 4 and query rows
[qg*512:(qg+1)*512] (qg = c % 4). The mask is packed into the sign of
A on the host (am = m ? a : -1), Hn ships pre-transposed fp16.

Device algorithm (per core):
  G    = (Wq^T Wk) / sqrt(D)  fp32 matmul -> fp16   [256,256] (PE)
  btab = ln(clip(a)*ge + tiny) - ln(1 - clip(a)*ge)  fp16     (DVE+ACT)
         (ge = mask recovered from the sign of am; masked -> -69)
  VT   = G^T HqT  per batch, fp16                   [256,512] (PE)
  S    = VT.T @ HnT + I.btab  (fp16 matmuls) PSUM f32         (PE)
  p1   = exp(S) -> SBUF bf16, accum rowsum rs                 (ACT)
  out  = p1 * (1/rs)  bf16 -> DRAM                            (DVE)

Emission order = per-engine execution order, so bias prep / VT builds
are interleaved between the first sweep's tiles to overlap the ramp.
Output is bf16 on device, upcast to f32 on host.
"""

import math

import numpy as np

import concourse.bass as bass
import concourse.bacc as bacc
import concourse.tile as tile
from concourse import mybir
from concourse import bass_utils

F32 = mybir.dt.float32
F32R = mybir.dt.float32r
BF16 = mybir.dt.bfloat16
FP16 = mybir.dt.float16

B_FULL = 16
N = 2048
D = 256
NBG = 2   # batch groups
NQG = 4   # seq (query-row) groups
NB = B_FULL // NBG        # batches per core = 8
NQ = N // NQG             # query rows per core = 512
NQT = NQ // 128           # q tiles per core = 4
EPS = 1e-3
SCALE = 1.0 / math.sqrt(float(D))  # 1/16

_CACHE = {}


def _patch_act_tables():
    # Prefer the activation-table set that holds BOTH Ln and Exp so the
    # scalar engine never reloads tables between bias-prep logs and
    # softmax exps.
    from concourse import hw_specs as _hw
    if getattr(_hw, "_combined_first", False):
        return
    _orig = _hw.get_activation_tables

    def _patched(module_arch):
        tabs = _orig(module_arch)
        pref = "natural_log_exp_and_others"
        if pref in tabs:
            both = {mybir.ActivationFunctionType.Ln,
                    mybir.ActivationFunctionType.Exp,
                    mybir.ActivationFunctionType.Copy}
            tabs = {
                k: (v if k == pref else (v - both))
                for k, v in tabs.items()
            }
        return tabs

    _hw.get_activation_tables = _patched
    import concourse.bacc as _bacc_mod
    _bacc_mod.get_activation_tables = _patched
    _hw._combined_first = True


def _build():
    _patch_act_tables()
    nc = bacc.Bacc("TRN2", debug=False, enable_asserts=False)

    hnt_d = nc.dram_tensor("hnt", [NB, D, N], FP16, kind="ExternalInput").ap()
    hqt_d = nc.dram_tensor("hqt", [NB, D, NQ], FP16, kind="ExternalInput").ap()
    am_d = nc.dram_tensor("am", [NQ, N], F32, kind="ExternalInput").ap()
    # wq/wk pre-packed on host to [p, c, d] so the load is contiguous
    wq_d = nc.dram_tensor("wq", [128, 2, D], F32, kind="ExternalInput").ap()
    wk_d = nc.dram_tensor("wk", [128, 2, D], F32, kind="ExternalInput").ap()
    idb_d = nc.dram_tensor("idb", [128, 128], FP16, kind="ExternalInput").ap()
    o_d = nc.dram_tensor("o", [NB, NQ, N], BF16, kind="ExternalOutput").ap()

    with tile.TileContext(nc) as tc:
        with (
            tc.tile_pool(name="consts", bufs=1) as consts,
            tc.tile_pool(name="amp", bufs=2) as amp,
            tc.tile_pool(name="prep", bufs=2) as prep,
            tc.tile_pool(name="bpool", bufs=1) as bpool,
            tc.tile_pool(name="hntp", bufs=10) as hntp,
            tc.tile_pool(name="hqtp", bufs=16) as hqtp,
            tc.tile_pool(name="vtp", bufs=16) as vtp,
            tc.tile_pool(name="pp", bufs=3) as pp,
            tc.tile_pool(name="op", bufs=3) as op_pool,
            tc.tile_pool(name="rsp", bufs=8) as rsp,
            tc.tile_pool(name="ps_s", bufs=2, space="PSUM") as ps_s,
        ):
            # ---- tiny const + ACT table preload (Ln/Exp/Copy set) ----
            tinyc = consts.tile([128, 1], F32, tag="tiny")
            nc.vector.memset(tinyc, 1e-30)
            wrm = consts.tile([128, 128], FP16, tag="wrm")
            nc.vector.memset(wrm, 0.5)
            warm = consts.tile([128, 1], F32, tag="warm")
            nc.scalar.activation(
                out=warm, in_=tinyc,
                func=mybir.ActivationFunctionType.Ln, bias=1.0, scale=1.0,
            )

            # ---- constants ----
            wq_sb = consts.tile([128, 2, D], F32, tag="wq")
            nc.sync.dma_start(out=wq_sb, in_=wq_d)
            wk_sb = consts.tile([128, 2, D], F32, tag="wk")
            nc.sync.dma_start(out=wk_sb, in_=wk_d)

            idb = consts.tile([128, 128], FP16, tag="idb")
            nc.sync.dma_start(out=idb, in_=idb_d)

            # am tiles (gpsimd queue so they land early, independent of
            # the sync ring that streams hqt/hnt); am[3] is emitted later
            # so it never blocks the SWDGE ring head on a buffer WAR.
            am_tiles = {}

            def emit_am(t):
                am_t = amp.tile([128, N], F32, tag="am", name=f"am{t}")
                nc.gpsimd.dma_start(out=am_t, in_=am_d[t * 128:(t + 1) * 128, :])
                am_tiles[t] = am_t

            for t in range(2):
                emit_am(t)

            def emit_hnt(b):
                hnt = []
                for i in range(2):
                    h_i = hntp.tile([128, N], FP16, tag="hnt", name=f"hnt{b}_{i}")
                    nc.sync.dma_start(
                        out=h_i, in_=hnt_d[b, i * 128:(i + 1) * 128, :]
                    )
                    hnt.append(h_i)
                return hnt

            def emit_hqt(b):
                hq = []
                for i in range(2):
                    hq_i = hqtp.tile([128, NQ], FP16, tag="hqt", name=f"hqt{b}_{i}")
                    nc.sync.dma_start(
                        out=hq_i, in_=hqt_d[b, i * 128:(i + 1) * 128, :]
                    )
                    hq.append(hq_i)
                return hq

            # preamble loads, interleaved on the FIFO sync ring so each
            # resource lands just before its consumer needs it
            hnts = {}
            hqts = {}
            hnts[0] = emit_hnt(0)
            hqts[0] = emit_hqt(0)
            hqts[1] = emit_hqt(1)
            hnts[1] = emit_hnt(1)
            hqts[2] = emit_hqt(2)
            hqts[3] = emit_hqt(3)

            # ---- G = (Wq^T Wk) * SCALE : [256,256] as 2 tiles ----
            # The first gp tile also hosts HAM warm-up matmuls: ~3.5us of
            # back-to-back PE work as soon as the weights land, so the
            # clock gate opens (1.2 -> 2.4 GHz) before the first score
            # matmuls issue. Their results are overwritten by the real
            # G matmuls (start=True).
            g = []
            for i in range(2):
                gp = ps_s.tile([128, N], F32, tag="s", name=f"gp{i}")
                if i == 0:
                    # depends only on a memset tile, not the weight DMAs,
                    # so the clock gate opens at ~10us regardless of DMA
                    # arrival jitter
                    for w in range(12):
                        nc.tensor.matmul(
                            gp[:, :128],
                            lhsT=wrm, rhs=wrm,
                            start=True, stop=True,
                        )
                for e in range(2):
                    nc.tensor.matmul(
                        gp[:, :D],
                        lhsT=wq_sb[:, e, i * 128:(i + 1) * 128],
                        rhs=wk_sb[:, e, :],
                        start=(e == 0),
                        stop=(e == 1),
                    )
                g_i = consts.tile([128, D], FP16, tag=f"g{i}", name=f"g{i}")
                nc.scalar.mul(out=g_i, in_=gp[:, :D], mul=SCALE)
                g.append(g_i)

            # ---- bias table prep (all DVE + ACT; gpsimd is too slow and
            # steals the DVE SBUF port) ----
            # btab[t] = ln(acm + tiny) - ln(1 - acm)   (f32r)
            # where ge = (am >= 0), acm = clip(am, eps, 1-eps) * ge
            # (masked entries: acm = 0 -> btab = ln(tiny) ~ -69)
            btab = []
            for t in range(NQT):
                btab.append(bpool.tile([128, N], FP16, tag=f"bt{t}", name=f"bt{t}"))

            def emit_bprep_pre(t):
                am_t = am_tiles[t]
                ge = prep.tile([128, N], FP16, tag="ge", name=f"ge{t}")
                nc.vector.tensor_scalar(
                    out=ge, in0=am_t, scalar1=0.0, scalar2=None,
                    op0=mybir.AluOpType.is_ge,
                )
                ac = prep.tile([128, N], F32, tag="ac", name=f"ac{t}")
                nc.vector.tensor_scalar(
                    out=ac, in0=am_t, scalar1=float(EPS),
                    scalar2=float(1.0 - EPS),
                    op0=mybir.AluOpType.max, op1=mybir.AluOpType.min,
                )
                acm = prep.tile([128, N], F32, tag="acm", name=f"acm{t}")
                nc.vector.tensor_mul(acm, ac, ge)
                la = prep.tile([128, N], F32, tag="la", name=f"la{t}")
                nc.scalar.activation(
                    out=la, in_=acm, func=mybir.ActivationFunctionType.Ln,
                    bias=tinyc, scale=1.0,
                )
                l1a = prep.tile([128, N], F32, tag="l1a", name=f"l1a{t}")
                nc.scalar.activation(
                    out=l1a, in_=acm, func=mybir.ActivationFunctionType.Ln,
                    bias=1.0, scale=-1.0,
                )
                return la, l1a

            def emit_bprep_sub(t, la, l1a):
                nc.vector.tensor_sub(btab[t], la, l1a)

            def emit_bprep(t):
                la, l1a = emit_bprep_pre(t)
                emit_bprep_sub(t, la, l1a)

            # ---- VT builder (copy engine selectable: DVE is the busy
            # engine during sweep A, so later VTs copy via ACT) ----
            def emit_vt(b, eng="dve"):
                vt = []
                for j in range(2):
                    vt_j = vtp.tile([128, NQ], FP16, tag="vt", name=f"vt{b}_{j}")
                    vp = ps_s.tile(
                        [128, N], F32, tag="s", name=f"vp{b}{j}"
                    )[:, :NQ]
                    for i in range(2):
                        nc.tensor.matmul(
                            vp,
                            lhsT=g[i][:, j * 128:(j + 1) * 128],
                            rhs=hqts[b][i],
                            start=(i == 0),
                            stop=(i == 1),
                        )
                    if eng == "act":
                        nc.scalar.copy(out=vt_j, in_=vp)
                    else:
                        nc.vector.tensor_copy(out=vt_j, in_=vp)
                    vt.append(vt_j)
                return vt

            vts = {}

            def emit_qtile_s(b, qt):
                qsl = slice(qt * 128, (qt + 1) * 128)
                vt, hnt = vts[b], hnts[b]
                s_ps = ps_s.tile([128, N], F32, tag="s", name=f"s{b}{qt}")
                for j in range(2):
                    for c in range(4):
                        csl = slice(c * 512, (c + 1) * 512)
                        nc.tensor.matmul(
                            s_ps[:, csl],
                            lhsT=vt[j][:, qsl],
                            rhs=hnt[j][:, csl],
                            start=(j == 0),
                            stop=False,
                        )
                return s_ps

            def emit_qtile_rest(b, qt, s_ps):
                qsl = slice(qt * 128, (qt + 1) * 128)
                for c in range(4):
                    csl = slice(c * 512, (c + 1) * 512)
                    nc.tensor.matmul(
                        s_ps[:, csl], lhsT=idb, rhs=btab[qt][:, csl],
                        start=False, stop=True,
                    )
                rs = rsp.tile([128, 1], F32, tag="rs", name=f"rs{b}{qt}")
                p1 = pp.tile([128, N], BF16, tag="p", name=f"p{b}{qt}")
                nc.scalar.activation(
                    out=p1, in_=s_ps,
                    func=mybir.ActivationFunctionType.Exp,
                    accum_out=rs,
                )
                rinv = rsp.tile([128, 1], F32, tag="rinv", name=f"ri{b}{qt}")
                nc.vector.reciprocal(out=rinv, in_=rs)
                out_t = op_pool.tile([128, N], BF16, tag="o", name=f"o{b}{qt}")
                nc.vector.tensor_scalar(
                    out=out_t, in0=p1, scalar1=rinv, scalar2=None,
                    op0=mybir.AluOpType.mult,
                )
                nc.gpsimd.dma_start(out=o_d[b, qsl, :], in_=out_t)

            def emit_qtile(b, qt):
                emit_qtile_rest(b, qt, emit_qtile_s(b, qt))

            hnts[2] = emit_hnt(2)
            hnts[3] = emit_hnt(3)
            for b in range(4, NB):
                hqts[b] = emit_hqt(b)

            # ---- sweep A: qt-major over batches 0..3. Per-engine
            # execution follows emission order: S matmuls of the first
            # tile pair are emitted before any bias matmuls (PE starts
            # as soon as vt/hnt land), the bprep(0) subtract is deferred
            # past the VT casts so the DVE FIFO never blocks them, and
            # each group's bias prep runs one qt group ahead.
            la0, l1a0 = emit_bprep_pre(0)
            vts[0] = emit_vt(0)
            vts[1] = emit_vt(1)
            s00 = emit_qtile_s(0, 0)
            s10 = emit_qtile_s(1, 0)
            vts[2] = emit_vt(2, "act")
            vts[3] = emit_vt(3, "act")
            emit_bprep_sub(0, la0, l1a0)
            emit_qtile_rest(0, 0, s00)
            emit_qtile_rest(1, 0, s10)
            emit_bprep(1)
            vts[4] = emit_vt(4, "act")
            emit_am(2)
            emit_qtile(2, 0)
            emit_qtile(3, 0)
            for qt in range(1, NQT):
                if qt < NQT - 1:
                    emit_bprep(qt + 1)
                    vts[qt + 4] = emit_vt(qt + 4, "act")
                if qt == 1:
                    emit_am(3)
                    hnts[4] = emit_hnt(4)
                if qt == 2:
                    vts[7] = emit_vt(7, "act")
                    hnts[5] = emit_hnt(5)
                for b in range(4):
                    emit_qtile(b, qt)

            # ---- sweep B: b-major over batches 4..7 ----
            for b in range(4, NB):
                if b + 2 < NB:
                    hnts[b + 2] = emit_hnt(b + 2)
                for qt in range(NQT):
                    emit_qtile(b, qt)
    nc.compile()
    return nc


def _get_nc():
    if "nc" not in _CACHE:
        _CACHE["nc"] = _build()
    return _CACHE["nc"]


def make_in_maps(Hn, A_stat, M_mask, Wq, Wk):
    Hn = np.ascontiguousarray(np.asarray(Hn, dtype=np.float32))
    A_stat = np.ascontiguousarray(np.asarray(A_stat, dtype=np.float32))
    M_mask = np.asarray(M_mask)
    Wq = np.ascontiguousarray(np.asarray(Wq, dtype=np.float32))
    Wk = np.ascontiguousarray(np.asarray(Wk, dtype=np.float32))
    assert Hn.shape == (B_FULL, N, D)

    # mask packed into the sign: masked entries become -1.0
    am_full = np.where(M_mask != 0, A_stat, np.float32(-1.0)).astype(np.float32)

    # [16, 256, 2048] transposed-node layout, fp16
    hnt_full = np.ascontiguousarray(Hn.astype(np.float16).transpose(0, 2, 1))

    in_maps = []
    for c in range(8):
        bg, qg = c // NQG, c % NQG
        bsl = slice(bg * NB, (bg + 1) * NB)
        qsl = slice(qg * NQ, (qg + 1) * NQ)
        in_maps.append({
            "hnt": hnt_full[bsl],
            "hqt": np.ascontiguousarray(hnt_full[bsl][:, :, qsl]),
            "am": np.ascontiguousarray(am_full[qsl]),
            "wq": np.ascontiguousarray(Wq.reshape(2, 128, D).transpose(1, 0, 2)),
            "wk": np.ascontiguousarray(Wk.reshape(2, 128, D).transpose(1, 0, 2)),
            "idb": np.eye(128, dtype=np.float16),
        })
    return in_maps


def assemble(results):
    out = np.empty((B_FULL, N, N), dtype=np.float32)
    for c in range(8):
        bg, qg = c // NQG, c % NQG
        o = results[c]["o"]
        out[bg * NB:(bg + 1) * NB, qg * NQ:(qg + 1) * NQ, :] = (
            np.asarray(o).astype(np.float32)
        )
    return out


def kernel(Hn, A_stat, M_mask, Wq, Wk):
    in_maps = make_in_maps(Hn, A_stat, M_mask, Wq, Wk)
    nc = _get_nc()
    res = bass_utils.run_bass_kernel_spmd(nc, in_maps, core_ids=list(range(8)))
    return assemble(res.results)


if __name__ == "__main__":
    rng = np.random.default_rng(0)
    inputs = {
        "Hn": rng.standard_normal((B_FULL, N, D), dtype=np.float32),
        "A_stat": rng.random((N, N), dtype=np.float32),
        "M_mask": rng.integers(0, 2, size=(N, N), dtype=np.int32),
        "Wq": rng.standard_normal((D, D), dtype=np.float32) / 16,
        "Wk": rng.standard_normal((D, D), dtype=np.float32) / 16,
    }
    out = kernel(**inputs)
    print(out.shape, out.dtype, out.sum())



# revision 24
# speedup vs baseline: 1.2594x; 1.2594x over previous
"""
Trainium2 Bass kernel for DynamicGraphAttention
(softmax(Hn Wq^T (Hn Wk^T)^T / sqrt(D) + eta*logit(clip(A)) masked)).

Shapes (hardcoded):
  Hn     [16, 2048, 256] f32
  A_stat [2048, 2048]    f32
  M_mask [2048, 2048]    int32
  Wq, Wk [256, 256]      f32
  out    [16, 2048, 2048] f32

Sharding across 8 NeuronCores: 4 batch-groups x 2 seq(query)-groups.
Core c handles batches of group bg = c // 2 (4 batches) and query rows
[qg*1024:(qg+1)*1024] (qg = c % 2). The program is identical on all
cores (SPMD): for qg=1 cores the host swaps the two key-column halves
of hnt and w so the core's own query block is always columns [0:1024],
and the assemble step swaps the output's key axis back.

Multiplicative-bias softmax: softmax(S + b) == exp(S)*w / sum(exp(S)*w)
with w = exp(eta*logit(clip(a)))*mask = mask * a/(1-a) (eta=1), computed
on the host in fp16 (masked entries are exactly 0, matching the
reference's -inf -> 0). This removes the PE bias matmuls, the ACT Ln
passes and the DVE bias prep of the additive formulation.

G = (Wq^T Wk)/sqrt(D) is folded host-side (fp16, [128,2,256]); Hn ships
pre-transposed fp16 and the query block is a column-slice view of it
(no separate hqt load).

Device algorithm (per core):
  VT   = G^T HqT  per batch, fp16          [256,1024]        (PE)
  S    = VT.T @ HnT  (fp16 matmuls) PSUM f32                 (PE)
  e    = exp(S) -> SBUF fp16                                 (ACT)
  pw   = e * w[qt], rs = rowsum(pw)   (tensor_tensor_reduce) (DVE)
  out  = pw * (1/rs)  fp16 -> DRAM                           (DVE)

Emission order = per-engine execution order; VT builds and w loads are
interleaved into the sweep. Output is fp16 on device, upcast on host.
"""

import math

import numpy as np

import concourse.bass as bass
import concourse.bacc as bacc
import concourse.tile as tile
from concourse import mybir
from concourse import bass_utils

F32 = mybir.dt.float32
FP16 = mybir.dt.float16
BF16 = mybir.dt.bfloat16

B_FULL = 16
N = 2048
D = 256
NBG = 4   # batch groups
NQG = 2   # seq (query-row) groups
NB = B_FULL // NBG        # batches per core = 4
NQ = N // NQG             # query rows per core = 1024
NQT = NQ // 128           # q tiles per core = 8
EPS = 1e-3
SCALE = 1.0 / math.sqrt(float(D))  # 1/16

# qt slots using the additive ln(w) bias via PE identity-matmul (the rest
# multiply by w on the DVE); qt slots whose normalize runs on ACT.
# Chosen to balance PE / ACT / DVE busy time.
ADD_QT = (1, 4, 6)
ACT_NORM_QT = (1, 4)

_CACHE = {}


def _patch_act_tables():
    # Prefer the activation-table set that holds Exp (+Copy) so the
    # scalar engine never reloads tables mid-kernel.
    from concourse import hw_specs as _hw
    if getattr(_hw, "_combined_first", False):
        return
    _orig = _hw.get_activation_tables

    def _patched(module_arch):
        tabs = _orig(module_arch)
        pref = "natural_log_exp_and_others"
        if pref in tabs:
            both = {mybir.ActivationFunctionType.Ln,
                    mybir.ActivationFunctionType.Exp,
                    mybir.ActivationFunctionType.Copy}
            tabs = {
                k: (v if k == pref else (v - both))
                for k, v in tabs.items()
            }
        return tabs

    _hw.get_activation_tables = _patched
    import concourse.bacc as _bacc_mod
    _bacc_mod.get_activation_tables = _patched
    _hw._combined_first = True


def _build():
    _patch_act_tables()
    nc = bacc.Bacc("TRN2", debug=False, enable_asserts=False)

    hnt_d = nc.dram_tensor("hnt", [NB, D, N], FP16, kind="ExternalInput").ap()
    # per-qt-slot bias sheet: ln(w) for ADD_QT slots, w for the others
    w_d = nc.dram_tensor("w", [NQT, 128, N], FP16, kind="ExternalInput").ap()
    g_d = nc.dram_tensor("g", [128, 2, D], FP16, kind="ExternalInput").ap()
    idb_d = nc.dram_tensor("idb", [128, 128], FP16, kind="ExternalInput").ap()
    o_d = nc.dram_tensor("o", [NB, NQ, N], BF16, kind="ExternalOutput").ap()

    with tile.TileContext(nc) as tc:
        with (
            tc.tile_pool(name="consts", bufs=1) as consts,
            tc.tile_pool(name="wp", bufs=NQT) as wp,
            tc.tile_pool(name="hntp", bufs=10) as hntp,
            tc.tile_pool(name="vtp", bufs=4) as vtp,
            tc.tile_pool(name="pp", bufs=3) as pp,
            tc.tile_pool(name="pwp", bufs=3) as pwp,
            tc.tile_pool(name="op", bufs=3) as op_pool,
            tc.tile_pool(name="rsp", bufs=8) as rsp,
            tc.tile_pool(name="ps_s", bufs=2, space="PSUM") as ps_s,
        ):
            # ---- tiny const + ACT table preload (Exp) ----
            tinyc = consts.tile([128, 1], F32, tag="tiny")
            nc.vector.memset(tinyc, 1e-30)
            wrm = consts.tile([128, 128], FP16, tag="wrm")
            nc.vector.memset(wrm, 0.5)
            warm = consts.tile([128, 1], F32, tag="warm")
            nc.scalar.activation(
                out=warm, in_=tinyc,
                func=mybir.ActivationFunctionType.Exp, bias=0.0, scale=1.0,
            )

            # ---- constants ----
            g_sb = consts.tile([128, 2, D], FP16, tag="g")
            nc.sync.dma_start(out=g_sb, in_=g_d)
            idb = consts.tile([128, 128], FP16, tag="idb")
            nc.sync.dma_start(out=idb, in_=idb_d)

            # w tiles (gpsimd queue; first two early, rest staggered)
            w_tiles = {}

            def emit_w(t):
                w_t = wp.tile([128, N], FP16, tag="w", name=f"w{t}")
                nc.gpsimd.dma_start(out=w_t, in_=w_d[t])
                w_tiles[t] = w_t

            emit_w(0)
            emit_w(1)

            # hnt[b] as 4 subtiles [i][h]: i = d'-row block, h = col half.
            # Half 0 is always the core's own query block (host-permuted),
            # loaded first so VT can start early.
            hnts = {}

            def emit_hnt(b, half=None):
                halves = [0, 1] if half is None else [half]
                if b not in hnts:
                    hnts[b] = [[None, None], [None, None]]
                for h in halves:
                    for i in range(2):
                        t = hntp.tile([128, NQ], FP16, tag="hnt",
                                      name=f"hnt{b}_{i}{h}")
                        nc.sync.dma_start(
                            out=t,
                            in_=hnt_d[b, i * 128:(i + 1) * 128,
                                      h * NQ:(h + 1) * NQ],
                        )
                        hnts[b][i][h] = t
                return hnts[b]

            # preamble loads: b0's q-half, then b0 rest + b1
            emit_hnt(0, half=0)

            # ---- PE clock-gate warmup: back-to-back matmuls that depend
            # only on a memset tile, so they issue immediately.
            wps = ps_s.tile([128, N], F32, tag="s", name="warmps")
            for _ in range(12):
                nc.tensor.matmul(
                    wps[:, :128], lhsT=wrm, rhs=wrm, start=True, stop=True,
                )

            emit_hnt(0, half=1)
            emit_hnt(1)

            # ---- VT builder: VT[d',q] = sum_d G[d,d'] HqT[d,q] ----
            # vp PSUM written in 512-col chunks (bank-sized matmuls);
            # copy engine selectable for load balancing.
            def emit_vt(b, eng="dve"):
                vt = []
                hq = [hnts[b][i][0] for i in range(2)]
                for j in range(2):
                    vt_j = vtp.tile([128, NQ], FP16, tag="vt", name=f"vt{b}_{j}")
                    vp = ps_s.tile(
                        [128, N], F32, tag="s", name=f"vp{b}{j}"
                    )[:, :NQ]
                    for c in range(2):
                        csl = slice(c * 512, (c + 1) * 512)
                        for i in range(2):
                            nc.tensor.matmul(
                                vp[:, csl],
                                lhsT=g_sb[:, i, j * 128:(j + 1) * 128],
                                rhs=hq[i][:, csl],
                                start=(i == 0),
                                stop=(i == 1),
                            )
                    if eng == "act":
                        nc.scalar.copy(out=vt_j, in_=vp)
                    elif eng == "gps":
                        nc.gpsimd.tensor_copy(out=vt_j, in_=vp)
                    else:
                        nc.vector.tensor_copy(out=vt_j, in_=vp)
                    vt.append(vt_j)
                return vt

            vts = {}

            def emit_qtile_s(b, qt):
                qsl = slice(qt * 128, (qt + 1) * 128)
                vt, hnt = vts[b], hnts[b]
                add_bias = qt in ADD_QT
                s_ps = ps_s.tile([128, N], F32, tag="s", name=f"s{b}{qt}")
                for j in range(2):
                    for c in range(4):
                        csl = slice(c * 512, (c + 1) * 512)
                        rhs = hnt[j][c // 2][:, (c % 2) * 512:(c % 2 + 1) * 512]
                        nc.tensor.matmul(
                            s_ps[:, csl],
                            lhsT=vt[j][:, qsl],
                            rhs=rhs,
                            start=(j == 0),
                            stop=(j == 1) and not add_bias,
                        )
                if add_bias:
                    # S += btab[qt] via identity matmul (PSUM accumulate)
                    for c in range(4):
                        csl = slice(c * 512, (c + 1) * 512)
                        nc.tensor.matmul(
                            s_ps[:, csl], lhsT=idb, rhs=w_tiles[qt][:, csl],
                            start=False, stop=True,
                        )
                return s_ps

            pending_norms = []

            def emit_norm(b, qt, pw, rinv, eng):
                qsl = slice(qt * 128, (qt + 1) * 128)
                out_t = op_pool.tile([128, N], BF16, tag="o", name=f"o{b}{qt}")
                if eng == "act":
                    nc.scalar.mul(out=out_t, in_=pw, mul=rinv)
                else:
                    nc.vector.tensor_scalar(
                        out=out_t, in0=pw, scalar1=rinv, scalar2=None,
                        op0=mybir.AluOpType.mult,
                    )
                nc.gpsimd.dma_start(out=o_d[b, qsl, :], in_=out_t)

            def flush_pending():
                while pending_norms:
                    emit_norm(*pending_norms.pop(0), "act")

            def emit_qtile_rest(b, qt, s_ps, norm_eng="dve"):
                add_bias = qt in ADD_QT
                p1 = pp.tile([128, N], FP16, tag="p", name=f"p{b}{qt}")
                rs = rsp.tile([128, 1], F32, tag="rs", name=f"rs{b}{qt}")
                if add_bias:
                    # bias already in S: exp accumulates the rowsum itself
                    nc.scalar.activation(
                        out=p1, in_=s_ps,
                        func=mybir.ActivationFunctionType.Exp,
                        accum_out=rs,
                    )
                    flush_pending()
                    pw = p1
                else:
                    nc.scalar.activation(
                        out=p1, in_=s_ps,
                        func=mybir.ActivationFunctionType.Exp,
                    )
                    # ACT-normalizes of older tiles go right behind this exp
                    flush_pending()
                    # pw = (p1*1+0)*w[qt], rs = rowsum (fused custom DVE op)
                    pw = pwp.tile([128, N], FP16, tag="pw", name=f"pw{b}{qt}")
                    nc.vector.affine_mul_reduce(
                        out=pw, accum_out=rs, in0=p1, in1=w_tiles[qt],
                        scale=1.0, bias=0.0,
                    )
                rinv = rsp.tile([128, 1], F32, tag="rinv", name=f"ri{b}{qt}")
                nc.vector.reciprocal(out=rinv, in_=rs)
                if norm_eng == "act":
                    pending_norms.append((b, qt, pw, rinv))
                else:
                    emit_norm(b, qt, pw, rinv, "dve")

            def emit_qtile(b, qt):
                eng = "act" if qt in ACT_NORM_QT else "dve"
                emit_qtile_rest(b, qt, emit_qtile_s(b, qt), eng)

            # ---- pipeline: batch-major, VT one batch ahead ----
            vts[0] = emit_vt(0)          # DVE copies (idle at ramp)
            s00 = emit_qtile_s(0, 0)
            emit_w(2)
            emit_w(3)
            emit_qtile_rest(0, 0, s00)
            emit_qtile(0, 1)
            vts[1] = emit_vt(1)
            emit_w(4)
            emit_qtile(0, 2)
            emit_w(5)
            emit_hnt(2)
            emit_qtile(0, 3)
            emit_w(6)
            emit_qtile(0, 4)
            emit_w(7)
            emit_qtile(0, 5)
            emit_qtile(0, 6)
            emit_qtile(0, 7)

            for b in range(1, NB):
                for qt in range(NQT):
                    if qt == 1 and b + 1 < NB:
                        vts[b + 1] = emit_vt(b + 1)
                    if qt == 3 and b + 2 < NB:
                        emit_hnt(b + 2)
                    emit_qtile(b, qt)
            flush_pending()
    nc.compile()
    return nc


def _get_nc():
    if "nc" not in _CACHE:
        _CACHE["nc"] = _build()
    return _CACHE["nc"]


def _swap_halves(x):
    # swap the two key-column halves (involution)
    return np.concatenate([x[..., NQ:], x[..., :NQ]], axis=-1)


def make_in_maps(Hn, A_stat, M_mask, Wq, Wk):
    Hn = np.ascontiguousarray(np.asarray(Hn, dtype=np.float32))
    A_stat = np.asarray(A_stat, dtype=np.float32)
    M_mask = np.asarray(M_mask)
    Wq = np.asarray(Wq, dtype=np.float32)
    Wk = np.asarray(Wk, dtype=np.float32)
    assert Hn.shape == (B_FULL, N, D)

    # multiplicative bias: w = mask * a/(1-a), a = clip(A, eps, 1-eps)
    # additive form (ADD_QT slots): btab = ln(w) = logit(a); masked -> -69
    a = np.clip(A_stat, EPS, 1.0 - EPS)
    mask = M_mask != 0
    w_full = np.where(mask, a / (1.0 - a), 0.0).astype(np.float16)
    btab_full = np.where(
        mask, np.log(a) - np.log1p(-a), np.float32(-69.0)
    ).astype(np.float16)

    # G = (Wq^T Wk)/sqrt(D), packed [p, i, e] = G[i*128+p, e]
    G = (Wq.T @ Wk) * SCALE
    g_packed = np.ascontiguousarray(
        G.reshape(2, 128, D).transpose(1, 0, 2).astype(np.float16)
    )

    # [16, 256, 2048] transposed-node layout, fp16
    hnt_full = np.ascontiguousarray(Hn.astype(np.float16).transpose(0, 2, 1))

    in_maps = []
    for c in range(8):
        bg, qg = c // NQG, c % NQG
        bsl = slice(bg * NB, (bg + 1) * NB)
        qsl = slice(qg * NQ, (qg + 1) * NQ)
        hnt_c = hnt_full[bsl]
        w_c = w_full[qsl].reshape(NQT, 128, N).copy()
        b_c = btab_full[qsl].reshape(NQT, 128, N)
        for t in ADD_QT:
            w_c[t] = b_c[t]
        if qg == 1:
            hnt_c = _swap_halves(hnt_c)
            w_c = _swap_halves(w_c)
        in_maps.append({
            "hnt": np.ascontiguousarray(hnt_c),
            "w": np.ascontiguousarray(w_c),
            "g": g_packed,
            "idb": np.eye(128, dtype=np.float16),
        })
    return in_maps


def assemble(results):
    out = np.empty((B_FULL, N, N), dtype=np.float32)
    for c in range(8):
        bg, qg = c // NQG, c % NQG
        o = np.asarray(results[c]["o"])
        if qg == 1:
            o = _swap_halves(o)
        out[bg * NB:(bg + 1) * NB, qg * NQ:(qg + 1) * NQ, :] = (
            o.astype(np.float32)
        )
    return out


def kernel(Hn, A_stat, M_mask, Wq, Wk):
    in_maps = make_in_maps(Hn, A_stat, M_mask, Wq, Wk)
    nc = _get_nc()
    res = bass_utils.run_bass_kernel_spmd(nc, in_maps, core_ids=list(range(8)))
    return assemble(res.results)


if __name__ == "__main__":
    rng = np.random.default_rng(0)
    inputs = {
        "Hn": rng.standard_normal((B_FULL, N, D), dtype=np.float32),
        "A_stat": rng.random((N, N), dtype=np.float32),
        "M_mask": rng.integers(0, 2, size=(N, N), dtype=np.int32),
        "Wq": rng.standard_normal((D, D), dtype=np.float32) / 16,
        "Wk": rng.standard_normal((D, D), dtype=np.float32) / 16,
    }
    out = kernel(**inputs)
    print(out.shape, out.dtype, out.sum())


# revision 29
# speedup vs baseline: 1.3362x; 1.0609x over previous
"""
Trainium2 Bass kernel for DynamicGraphAttention
(softmax(Hn Wq^T (Hn Wk^T)^T / sqrt(D) + eta*logit(clip(A)) masked)).

Shapes (hardcoded):
  Hn     [16, 2048, 256] f32
  A_stat [2048, 2048]    f32
  M_mask [2048, 2048]    int32
  Wq, Wk [256, 256]      f32
  out    [16, 2048, 2048] f32

Sharding across 8 NeuronCores: 4 batch-groups x 2 seq(query)-groups.
Core c handles batches of group bg = c // 2 (4 batches) and query rows
[qg*1024:(qg+1)*1024] (qg = c % 2). The program is identical on all
cores (SPMD): for qg=1 cores the host swaps the two key-column halves
of hnt and w so the core's own query block is always columns [0:1024],
and the assemble step swaps the output's key axis back.

Multiplicative-bias softmax: softmax(S + b) == exp(S)*w / sum(exp(S)*w)
with w = exp(eta*logit(clip(a)))*mask = mask * a/(1-a) (eta=1), computed
on the host in fp16 (masked entries are exactly 0, matching the
reference's -inf -> 0). This removes the PE bias matmuls, the ACT Ln
passes and the DVE bias prep of the additive formulation.

G = (Wq^T Wk)/sqrt(D) is folded host-side (fp16, [128,2,256]); Hn ships
pre-transposed fp16 and the query block is a column-slice view of it
(no separate hqt load).

Device algorithm (per core):
  VT   = G^T HqT  per batch, fp16          [256,1024]        (PE)
  S    = VT.T @ HnT  (fp16 matmuls) PSUM f32                 (PE)
  e    = exp(S) -> SBUF fp16                                 (ACT)
  pw   = e * w[qt], rs = rowsum(pw)   (tensor_tensor_reduce) (DVE)
  out  = pw * (1/rs)  fp16 -> DRAM                           (DVE)

Emission order = per-engine execution order; VT builds and w loads are
interleaved into the sweep. Output is fp16 on device, upcast on host.
"""

import math

import numpy as np

import concourse.bass as bass
import concourse.bacc as bacc
import concourse.tile as tile
from concourse import mybir
from concourse import bass_utils

F32 = mybir.dt.float32
FP16 = mybir.dt.float16
BF16 = mybir.dt.bfloat16

B_FULL = 16
N = 2048
D = 256
NBG = 4   # batch groups
NQG = 2   # seq (query-row) groups
NB = B_FULL // NBG        # batches per core = 4
NQ = N // NQG             # query rows per core = 1024
NQT = NQ // 128           # q tiles per core = 8
EPS = 1e-3
SCALE = 1.0 / math.sqrt(float(D))  # 1/16

# qt slots using the additive ln(w) bias via PE identity-matmul (the rest
# multiply by w on the DVE); qt slots whose normalize runs on ACT.
# Chosen to balance PE / ACT / DVE busy time.
ADD_QT = (1, 4, 6)
ACT_NORM_QT = ()

_CACHE = {}


def _patch_act_tables():
    # Prefer the activation-table set that holds Exp (+Copy) so the
    # scalar engine never reloads tables mid-kernel.
    from concourse import hw_specs as _hw
    if getattr(_hw, "_combined_first", False):
        return
    _orig = _hw.get_activation_tables

    def _patched(module_arch):
        tabs = _orig(module_arch)
        pref = "natural_log_exp_and_others"
        if pref in tabs:
            both = {mybir.ActivationFunctionType.Ln,
                    mybir.ActivationFunctionType.Exp,
                    mybir.ActivationFunctionType.Copy}
            tabs = {
                k: (v if k == pref else (v - both))
                for k, v in tabs.items()
            }
        return tabs

    _hw.get_activation_tables = _patched
    import concourse.bacc as _bacc_mod
    _bacc_mod.get_activation_tables = _patched
    _hw._combined_first = True


def _build():
    _patch_act_tables()
    nc = bacc.Bacc("TRN2", debug=False, enable_asserts=False)

    hnt_d = nc.dram_tensor("hnt", [NB, D, N], FP16, kind="ExternalInput").ap()
    # per-qt-slot bias sheet: ln(w) for ADD_QT slots, w for the others
    w_d = nc.dram_tensor("w", [NQT, 128, N], FP16, kind="ExternalInput").ap()
    g_d = nc.dram_tensor("g", [128, 2, D], FP16, kind="ExternalInput").ap()
    idb_d = nc.dram_tensor("idb", [128, 128], FP16, kind="ExternalInput").ap()
    o_d = nc.dram_tensor("o", [NB, NQ, N], BF16, kind="ExternalOutput").ap()

    with tile.TileContext(nc) as tc:
        with (
            tc.tile_pool(name="consts", bufs=1) as consts,
            tc.tile_pool(name="wp", bufs=NQT) as wp,
            tc.tile_pool(name="hntp", bufs=12) as hntp,
            tc.tile_pool(name="vtp", bufs=4) as vtp,
            tc.tile_pool(name="pp", bufs=3) as pp,
            tc.tile_pool(name="pwp", bufs=3) as pwp,
            tc.tile_pool(name="op", bufs=3) as op_pool,
            tc.tile_pool(name="rsp", bufs=8) as rsp,
            tc.tile_pool(name="ps_s", bufs=2, space="PSUM") as ps_s,
        ):
            # ---- tiny const + ACT table preload (Exp) ----
            tinyc = consts.tile([128, 1], F32, tag="tiny")
            nc.vector.memset(tinyc, 1e-30)
            wrm = consts.tile([128, 128], FP16, tag="wrm")
            nc.vector.memset(wrm, 0.5)
            warm = consts.tile([128, 1], F32, tag="warm")
            nc.scalar.activation(
                out=warm, in_=tinyc,
                func=mybir.ActivationFunctionType.Exp, bias=0.0, scale=1.0,
            )

            # ---- constants ----
            g_sb = consts.tile([128, 2, D], FP16, tag="g")
            nc.sync.dma_start(out=g_sb, in_=g_d)
            idb = consts.tile([128, 128], FP16, tag="idb")
            nc.sync.dma_start(out=idb, in_=idb_d)

            # DMA rings: sync + act are HWDGE, gps is SWDGE (also carries
            # the output stores). Ramp loads are split across all three.
            rings = {"sync": nc.sync, "act": nc.scalar, "gps": nc.gpsimd}

            # w tiles
            w_tiles = {}

            def emit_w(t, ring="gps"):
                w_t = wp.tile([128, N], FP16, tag="w", name=f"w{t}")
                rings[ring].dma_start(out=w_t, in_=w_d[t])
                w_tiles[t] = w_t

            # hnt[b] as 4 subtiles [i][h]: i = d'-row block, h = col half.
            # Half 0 is always the core's own query block (host-permuted),
            # loaded first so VT can start early.
            hnts = {}

            def emit_hnt(b, half=None, ring="sync"):
                halves = [0, 1] if half is None else [half]
                if b not in hnts:
                    hnts[b] = [[None, None], [None, None]]
                for h in halves:
                    for i in range(2):
                        t = hntp.tile([128, NQ], FP16, tag="hnt",
                                      name=f"hnt{b}_{i}{h}")
                        rings[ring].dma_start(
                            out=t,
                            in_=hnt_d[b, i * 128:(i + 1) * 128,
                                      h * NQ:(h + 1) * NQ],
                        )
                        hnts[b][i][h] = t
                return hnts[b]

            # preamble loads spread over the three rings so batch 0+1 and
            # the first bias sheets land as fast as possible
            emit_w(1, "gps")
            emit_w(0, "gps")
            emit_hnt(0, half=0, ring="sync")
            emit_hnt(0, half=1, ring="act")

            # ---- PE clock-gate warmup: back-to-back matmuls that depend
            # only on a memset tile, so they issue immediately.
            wps = ps_s.tile([128, N], F32, tag="s", name="warmps")
            for _ in range(12):
                nc.tensor.matmul(
                    wps[:, :128], lhsT=wrm, rhs=wrm, start=True, stop=True,
                )

            emit_hnt(1, half=0, ring="act")
            emit_hnt(1, half=1, ring="sync")
            emit_w(2, "gps")
            emit_w(3, "gps")
            emit_w(6, "act")
            emit_w(7, "act")

            # ---- VT builder: VT[d',q] = sum_d G[d,d'] HqT[d,q] ----
            # vp PSUM written in 512-col chunks (bank-sized matmuls);
            # copy engine selectable for load balancing.
            def emit_vt(b, engs=("dve", "dve")):
                vt = []
                hq = [hnts[b][i][0] for i in range(2)]
                for j in range(2):
                    vt_j = vtp.tile([128, NQ], FP16, tag="vt", name=f"vt{b}_{j}")
                    vp = ps_s.tile(
                        [128, N], F32, tag="s", name=f"vp{b}{j}"
                    )[:, :NQ]
                    for c in range(2):
                        csl = slice(c * 512, (c + 1) * 512)
                        for i in range(2):
                            nc.tensor.matmul(
                                vp[:, csl],
                                lhsT=g_sb[:, i, j * 128:(j + 1) * 128],
                                rhs=hq[i][:, csl],
                                start=(i == 0),
                                stop=(i == 1),
                            )
                    if engs[j] == "act":
                        nc.scalar.copy(out=vt_j, in_=vp)
                    else:
                        nc.vector.tensor_copy(out=vt_j, in_=vp)
                    vt.append(vt_j)
                return vt

            vts = {}

            def emit_qtile_s(b, qt):
                qsl = slice(qt * 128, (qt + 1) * 128)
                vt, hnt = vts[b], hnts[b]
                add_bias = qt in ADD_QT
                s_ps = ps_s.tile([128, N], F32, tag="s", name=f"s{b}{qt}")
                for j in range(2):
                    for c in range(4):
                        csl = slice(c * 512, (c + 1) * 512)
                        rhs = hnt[j][c // 2][:, (c % 2) * 512:(c % 2 + 1) * 512]
                        nc.tensor.matmul(
                            s_ps[:, csl],
                            lhsT=vt[j][:, qsl],
                            rhs=rhs,
                            start=(j == 0),
                            stop=(j == 1) and not add_bias,
                        )
                if add_bias:
                    # S += btab[qt] via identity matmul (PSUM accumulate)
                    for c in range(4):
                        csl = slice(c * 512, (c + 1) * 512)
                        nc.tensor.matmul(
                            s_ps[:, csl], lhsT=idb, rhs=w_tiles[qt][:, csl],
                            start=False, stop=True,
                        )
                return s_ps

            pending_norms = []

            def emit_norm(b, qt, pw, rinv, eng):
                qsl = slice(qt * 128, (qt + 1) * 128)
                out_t = op_pool.tile([128, N], BF16, tag="o", name=f"o{b}{qt}")
                if eng == "act":
                    nc.scalar.mul(out=out_t, in_=pw, mul=rinv)
                else:
                    nc.vector.tensor_scalar(
                        out=out_t, in0=pw, scalar1=rinv, scalar2=None,
                        op0=mybir.AluOpType.mult,
                    )
                nc.gpsimd.dma_start(out=o_d[b, qsl, :], in_=out_t)

            def flush_pending():
                while pending_norms:
                    emit_norm(*pending_norms.pop(0), "act")

            def emit_qtile_rest(b, qt, s_ps, norm_eng="dve"):
                add_bias = qt in ADD_QT
                p1 = pp.tile([128, N], FP16, tag="p", name=f"p{b}{qt}")
                rs = rsp.tile([128, 1], F32, tag="rs", name=f"rs{b}{qt}")
                if add_bias:
                    # bias already in S: exp accumulates the rowsum itself
                    nc.scalar.activation(
                        out=p1, in_=s_ps,
                        func=mybir.ActivationFunctionType.Exp,
                        accum_out=rs,
                    )
                    flush_pending()
                    pw = p1
                else:
                    nc.scalar.activation(
                        out=p1, in_=s_ps,
                        func=mybir.ActivationFunctionType.Exp,
                    )
                    # ACT-normalizes of older tiles go right behind this exp
                    flush_pending()
                    # pw = (p1*1+0)*w[qt], rs = rowsum (fused custom DVE op)
                    pw = pwp.tile([128, N], FP16, tag="pw", name=f"pw{b}{qt}")
                    nc.vector.affine_mul_reduce(
                        out=pw, accum_out=rs, in0=p1, in1=w_tiles[qt],
                        scale=1.0, bias=0.0,
                    )
                rinv = rsp.tile([128, 1], F32, tag="rinv", name=f"ri{b}{qt}")
                nc.vector.reciprocal(out=rinv, in_=rs)
                if norm_eng == "act":
                    pending_norms.append((b, qt, pw, rinv))
                else:
                    emit_norm(b, qt, pw, rinv, "dve")

            def emit_qtile(b, qt):
                eng = "act" if qt in ACT_NORM_QT else "dve"
                emit_qtile_rest(b, qt, emit_qtile_s(b, qt), eng)

            # ---- pipeline: batch-major, VT(b+1) built just after (b, qt1)
            # (adjacent vp pair keeps the PSUM ring parity intact) ----
            vts[0] = emit_vt(0)          # DVE copies (idle at ramp)
            s00 = emit_qtile_s(0, 0)
            emit_w(4, "gps")
            emit_qtile_rest(0, 0, s00)
            emit_qtile(0, 1)
            vts[1] = emit_vt(1, ("act", "dve"))
            emit_w(5, "gps")
            emit_qtile(0, 2)
            emit_hnt(2)
            emit_qtile(0, 3)
            emit_qtile(0, 4)
            emit_qtile(0, 5)
            emit_qtile(0, 6)
            emit_qtile(0, 7)

            for b in range(1, NB):
                for qt in range(NQT):
                    emit_qtile(b, qt)
                    if qt == 1 and b + 1 < NB:
                        vts[b + 1] = emit_vt(b + 1, ("act", "dve"))
                    if qt == 3 and b + 2 < NB:
                        emit_hnt(b + 2)
            flush_pending()
    nc.compile()
    return nc


def _get_nc():
    if "nc" not in _CACHE:
        _CACHE["nc"] = _build()
    return _CACHE["nc"]


def _swap_halves(x):
    # swap the two key-column halves (involution)
    return np.concatenate([x[..., NQ:], x[..., :NQ]], axis=-1)


def make_in_maps(Hn, A_stat, M_mask, Wq, Wk):
    Hn = np.ascontiguousarray(np.asarray(Hn, dtype=np.float32))
    A_stat = np.asarray(A_stat, dtype=np.float32)
    M_mask = np.asarray(M_mask)
    Wq = np.asarray(Wq, dtype=np.float32)
    Wk = np.asarray(Wk, dtype=np.float32)
    assert Hn.shape == (B_FULL, N, D)

    # multiplicative bias: w = mask * a/(1-a), a = clip(A, eps, 1-eps)
    # additive form (ADD_QT slots): btab = ln(w) = logit(a); masked -> -69
    a = np.clip(A_stat, EPS, 1.0 - EPS)
    mask = M_mask != 0
    w_full = np.where(mask, a / (1.0 - a), 0.0).astype(np.float16)
    btab_full = np.where(
        mask, np.log(a) - np.log1p(-a), np.float32(-69.0)
    ).astype(np.float16)

    # G = (Wq^T Wk)/sqrt(D), packed [p, i, e] = G[i*128+p, e]
    G = (Wq.T @ Wk) * SCALE
    g_packed = np.ascontiguousarray(
        G.reshape(2, 128, D).transpose(1, 0, 2).astype(np.float16)
    )

    # [16, 256, 2048] transposed-node layout, fp16
    hnt_full = np.ascontiguousarray(Hn.astype(np.float16).transpose(0, 2, 1))

    in_maps = []
    for c in range(8):
        bg, qg = c // NQG, c % NQG
        bsl = slice(bg * NB, (bg + 1) * NB)
        qsl = slice(qg * NQ, (qg + 1) * NQ)
        hnt_c = hnt_full[bsl]
        w_c = w_full[qsl].reshape(NQT, 128, N).copy()
        b_c = btab_full[qsl].reshape(NQT, 128, N)
        for t in ADD_QT:
            w_c[t] = b_c[t]
        if qg == 1:
            hnt_c = _swap_halves(hnt_c)
            w_c = _swap_halves(w_c)
        in_maps.append({
            "hnt": np.ascontiguousarray(hnt_c),
            "w": np.ascontiguousarray(w_c),
            "g": g_packed,
            "idb": np.eye(128, dtype=np.float16),
        })
    return in_maps


def assemble(results):
    out = np.empty((B_FULL, N, N), dtype=np.float32)
    for c in range(8):
        bg, qg = c // NQG, c % NQG
        o = np.asarray(results[c]["o"])
        if qg == 1:
            o = _swap_halves(o)
        out[bg * NB:(bg + 1) * NB, qg * NQ:(qg + 1) * NQ, :] = (
            o.astype(np.float32)
        )
    return out


def kernel(Hn, A_stat, M_mask, Wq, Wk):
    in_maps = make_in_maps(Hn, A_stat, M_mask, Wq, Wk)
    nc = _get_nc()
    res = bass_utils.run_bass_kernel_spmd(nc, in_maps, core_ids=list(range(8)))
    return assemble(res.results)


if __name__ == "__main__":
    rng = np.random.default_rng(0)
    inputs = {
        "Hn": rng.standard_normal((B_FULL, N, D), dtype=np.float32),
        "A_stat": rng.random((N, N), dtype=np.float32),
        "M_mask": rng.integers(0, 2, size=(N, N), dtype=np.int32),
        "Wq": rng.standard_normal((D, D), dtype=np.float32) / 16,
        "Wk": rng.standard_normal((D, D), dtype=np.float32) / 16,
    }
    out = kernel(**inputs)
    print(out.shape, out.dtype, out.sum())


# revision 32
# speedup vs baseline: 1.4731x; 1.1025x over previous
"""
Trainium2 Bass kernel for DynamicGraphAttention
(softmax(Hn Wq^T (Hn Wk^T)^T / sqrt(D) + eta*logit(clip(A)) masked)).

Shapes (hardcoded):
  Hn     [16, 2048, 256] f32
  A_stat [2048, 2048]    f32
  M_mask [2048, 2048]    int32
  Wq, Wk [256, 256]      f32
  out    [16, 2048, 2048] f32

Sharding across 8 NeuronCores: 4 batch-groups x 2 seq(query)-groups.
Core c handles batches of group bg = c // 2 (4 batches) and query rows
[qg*1024:(qg+1)*1024] (qg = c % 2). The program is identical on all
cores (SPMD): for qg=1 cores the host swaps the two key-column halves
of hnt and w so the core's own query block is always columns [0:1024],
and the assemble step swaps the output's key axis back.

Multiplicative-bias softmax: softmax(S + b) == exp(S)*w / sum(exp(S)*w)
with w = exp(eta*logit(clip(a)))*mask = mask * a/(1-a) (eta=1), computed
on the host in fp16 (masked entries are exactly 0, matching the
reference's -inf -> 0). This removes the PE bias matmuls, the ACT Ln
passes and the DVE bias prep of the additive formulation.

G = (Wq^T Wk)/sqrt(D) is folded host-side (fp16, [128,2,256]); Hn ships
pre-transposed fp16 and the query block is a column-slice view of it
(no separate hqt load).

Device algorithm (per core):
  VT   = G^T HqT  per batch, fp16          [256,1024]        (PE)
  S    = VT.T @ HnT  (fp16 matmuls) PSUM f32                 (PE)
  e    = exp(S) -> SBUF fp16                                 (ACT)
  pw   = e * w[qt], rs = rowsum(pw)   (tensor_tensor_reduce) (DVE)
  out  = pw * (1/rs)  fp16 -> DRAM                           (DVE)

Emission order = per-engine execution order; VT builds and w loads are
interleaved into the sweep. Output is fp16 on device, upcast on host.
"""

import math

import numpy as np

import concourse.bass as bass
import concourse.bacc as bacc
import concourse.tile as tile
from concourse import mybir
from concourse import bass_utils

F32 = mybir.dt.float32
FP16 = mybir.dt.float16
BF16 = mybir.dt.bfloat16

B_FULL = 16
N = 2048
D = 256
NBG = 4   # batch groups
NQG = 2   # seq (query-row) groups
NB = B_FULL // NBG        # batches per core = 4
NQ = N // NQG             # query rows per core = 1024
NQT = NQ // 128           # q tiles per core = 8
EPS = 1e-3
SCALE = 1.0 / math.sqrt(float(D))  # 1/16

# qt slots using the additive ln(w) bias via PE identity-matmul (the rest
# multiply by w on the DVE); qt slots whose normalize runs on ACT.
# Chosen to balance PE / ACT / DVE busy time.
ADD_QT = (1, 4, 7)
ACT_NORM_QT = ()

_CACHE = {}


def _patch_act_tables():
    # Prefer the activation-table set that holds Exp (+Copy) so the
    # scalar engine never reloads tables mid-kernel.
    from concourse import hw_specs as _hw
    if getattr(_hw, "_combined_first", False):
        return
    _orig = _hw.get_activation_tables

    def _patched(module_arch):
        tabs = _orig(module_arch)
        pref = "natural_log_exp_and_others"
        if pref in tabs:
            both = {mybir.ActivationFunctionType.Ln,
                    mybir.ActivationFunctionType.Exp,
                    mybir.ActivationFunctionType.Copy}
            tabs = {
                k: (v if k == pref else (v - both))
                for k, v in tabs.items()
            }
        return tabs

    _hw.get_activation_tables = _patched
    import concourse.bacc as _bacc_mod
    _bacc_mod.get_activation_tables = _patched
    _hw._combined_first = True


def _build():
    _patch_act_tables()
    nc = bacc.Bacc("TRN2", debug=False, enable_asserts=False)

    hnt_d = nc.dram_tensor("hnt", [NB, D, N], FP16, kind="ExternalInput").ap()
    # per-qt-slot bias sheet: ln(w) for ADD_QT slots, w for the others
    w_d = nc.dram_tensor("w", [NQT, 128, N], FP16, kind="ExternalInput").ap()
    g_d = nc.dram_tensor("g", [128, 2, D], FP16, kind="ExternalInput").ap()
    idb_d = nc.dram_tensor("idb", [128, 128], FP16, kind="ExternalInput").ap()
    o_d = nc.dram_tensor("o", [NB, NQ, N], BF16, kind="ExternalOutput").ap()

    with tile.TileContext(nc) as tc:
        with (
            tc.tile_pool(name="consts", bufs=1) as consts,
            tc.tile_pool(name="wp", bufs=NQT) as wp,
            tc.tile_pool(name="hntp", bufs=12) as hntp,
            tc.tile_pool(name="vtp", bufs=4) as vtp,
            tc.tile_pool(name="pp", bufs=3) as pp,
            tc.tile_pool(name="pwp", bufs=3) as pwp,
            tc.tile_pool(name="op", bufs=3) as op_pool,
            tc.tile_pool(name="rsp", bufs=8) as rsp,
            tc.tile_pool(name="ps_s", bufs=2, space="PSUM") as ps_s,
        ):
            # ---- tiny const + ACT table preload (Exp) ----
            tinyc = consts.tile([128, 1], F32, tag="tiny")
            nc.vector.memset(tinyc, 1e-30)
            wrm = consts.tile([128, 128], FP16, tag="wrm")
            nc.vector.memset(wrm, 0.5)
            warm = consts.tile([128, 1], F32, tag="warm")
            nc.scalar.activation(
                out=warm, in_=tinyc,
                func=mybir.ActivationFunctionType.Exp, bias=0.0, scale=1.0,
            )

            # ---- constants ----
            g_sb = consts.tile([128, 2, D], FP16, tag="g")
            nc.sync.dma_start(out=g_sb, in_=g_d)
            idb = consts.tile([128, 128], FP16, tag="idb")
            nc.sync.dma_start(out=idb, in_=idb_d)

            # DMA rings: sync + act are HWDGE, gps is SWDGE (also carries
            # the output stores). Ramp loads are split across all three.
            rings = {"sync": nc.sync, "act": nc.scalar, "gps": nc.gpsimd}

            # w tiles
            w_tiles = {}

            def emit_w(t, ring="gps"):
                w_t = wp.tile([128, N], FP16, tag="w", name=f"w{t}")
                rings[ring].dma_start(out=w_t, in_=w_d[t])
                w_tiles[t] = w_t

            # hnt[b] as 4 subtiles [i][h]: i = d'-row block, h = col half.
            # Half 0 is always the core's own query block (host-permuted),
            # loaded first so VT can start early.
            hnts = {}

            def emit_hnt(b, half=None, ring="sync"):
                halves = [0, 1] if half is None else [half]
                if b not in hnts:
                    hnts[b] = [[None, None], [None, None]]
                for h in halves:
                    for i in range(2):
                        t = hntp.tile([128, NQ], FP16, tag="hnt",
                                      name=f"hnt{b}_{i}{h}")
                        rings[ring].dma_start(
                            out=t,
                            in_=hnt_d[b, i * 128:(i + 1) * 128,
                                      h * NQ:(h + 1) * NQ],
                        )
                        hnts[b][i][h] = t
                return hnts[b]

            # preamble loads spread over the three rings so batch 0+1 and
            # the first bias sheets land as fast as possible; the gpsimd
            # SWDGE ring is the fastest, so b0's query half rides it first
            emit_hnt(0, half=0, ring="gps")
            emit_hnt(0, half=1, ring="act")
            emit_w(1, "gps")
            emit_w(0, "gps")

            # ---- PE clock-gate warmup: back-to-back matmuls that depend
            # only on a memset tile, so they issue immediately and bridge
            # the input-DMA wait without losing the boost clock.
            wps = ps_s.tile([128, N], F32, tag="s", name="warmps")
            for _ in range(24):
                nc.tensor.matmul(
                    wps[:, :128], lhsT=wrm, rhs=wrm, start=True, stop=True,
                )

            emit_hnt(1, half=0, ring="sync")
            emit_hnt(1, half=1, ring="sync")
            emit_w(2, "gps")
            emit_w(3, "gps")
            emit_w(6, "act")
            emit_w(7, "act")

            # ---- VT builder: VT[d',q] = sum_d G[d,d'] HqT[d,q] ----
            # vp PSUM written in 512-col chunks (bank-sized matmuls);
            # copy engine selectable for load balancing.
            def emit_vt(b, engs=("dve", "dve")):
                vt = []
                hq = [hnts[b][i][0] for i in range(2)]
                for j in range(2):
                    vt_j = vtp.tile([128, NQ], FP16, tag="vt", name=f"vt{b}_{j}")
                    vp = ps_s.tile(
                        [128, N], F32, tag="s", name=f"vp{b}{j}"
                    )[:, :NQ]
                    for c in range(2):
                        csl = slice(c * 512, (c + 1) * 512)
                        for i in range(2):
                            nc.tensor.matmul(
                                vp[:, csl],
                                lhsT=g_sb[:, i, j * 128:(j + 1) * 128],
                                rhs=hq[i][:, csl],
                                start=(i == 0),
                                stop=(i == 1),
                            )
                    if engs[j] == "act":
                        nc.scalar.copy(out=vt_j, in_=vp)
                    else:
                        nc.vector.tensor_copy(out=vt_j, in_=vp)
                    vt.append(vt_j)
                return vt

            vts = {}

            def emit_qtile_s(b, qt):
                qsl = slice(qt * 128, (qt + 1) * 128)
                vt, hnt = vts[b], hnts[b]
                add_bias = qt in ADD_QT
                s_ps = ps_s.tile([128, N], F32, tag="s", name=f"s{b}{qt}")
                for j in range(2):
                    for c in range(4):
                        csl = slice(c * 512, (c + 1) * 512)
                        rhs = hnt[j][c // 2][:, (c % 2) * 512:(c % 2 + 1) * 512]
                        nc.tensor.matmul(
                            s_ps[:, csl],
                            lhsT=vt[j][:, qsl],
                            rhs=rhs,
                            start=(j == 0),
                            stop=(j == 1) and not add_bias,
                        )
                if add_bias:
                    # S += btab[qt] via identity matmul (PSUM accumulate)
                    for c in range(4):
                        csl = slice(c * 512, (c + 1) * 512)
                        nc.tensor.matmul(
                            s_ps[:, csl], lhsT=idb, rhs=w_tiles[qt][:, csl],
                            start=False, stop=True,
                        )
                return s_ps

            pending_norms = []

            def emit_norm(b, qt, pw, rinv, eng):
                qsl = slice(qt * 128, (qt + 1) * 128)
                out_t = op_pool.tile([128, N], BF16, tag="o", name=f"o{b}{qt}")
                if eng == "act":
                    nc.scalar.mul(out=out_t, in_=pw, mul=rinv)
                else:
                    nc.vector.tensor_scalar(
                        out=out_t, in0=pw, scalar1=rinv, scalar2=None,
                        op0=mybir.AluOpType.mult,
                    )
                nc.gpsimd.dma_start(out=o_d[b, qsl, :], in_=out_t)

            def flush_pending():
                while pending_norms:
                    emit_norm(*pending_norms.pop(0), "act")

            def emit_qtile_rest(b, qt, s_ps, norm_eng="dve"):
                add_bias = qt in ADD_QT
                p1 = pp.tile([128, N], FP16, tag="p", name=f"p{b}{qt}")
                rs = rsp.tile([128, 1], F32, tag="rs", name=f"rs{b}{qt}")
                if add_bias:
                    # bias already in S: exp accumulates the rowsum itself
                    nc.scalar.activation(
                        out=p1, in_=s_ps,
                        func=mybir.ActivationFunctionType.Exp,
                        accum_out=rs,
                    )
                    flush_pending()
                    pw = p1
                else:
                    nc.scalar.activation(
                        out=p1, in_=s_ps,
                        func=mybir.ActivationFunctionType.Exp,
                    )
                    # ACT-normalizes of older tiles go right behind this exp
                    flush_pending()
                    # pw = (p1*1+0)*w[qt], rs = rowsum (fused custom DVE op)
                    pw = pwp.tile([128, N], FP16, tag="pw", name=f"pw{b}{qt}")
                    nc.vector.affine_mul_reduce(
                        out=pw, accum_out=rs, in0=p1, in1=w_tiles[qt],
                        scale=1.0, bias=0.0,
                    )
                rinv = rsp.tile([128, 1], F32, tag="rinv", name=f"ri{b}{qt}")
                nc.vector.reciprocal(out=rinv, in_=rs)
                if norm_eng == "act":
                    pending_norms.append((b, qt, pw, rinv))
                else:
                    emit_norm(b, qt, pw, rinv, "dve")

            def emit_qtile(b, qt):
                eng = "act" if qt in ACT_NORM_QT else "dve"
                emit_qtile_rest(b, qt, emit_qtile_s(b, qt), eng)

            # ---- pipeline: batch-major, VT(b+1) built just after (b, qt1)
            # (adjacent vp pair keeps the PSUM ring parity intact) ----
            vts[0] = emit_vt(0)          # DVE copies (idle at ramp)
            s00 = emit_qtile_s(0, 0)
            emit_w(4, "gps")
            emit_qtile_rest(0, 0, s00)
            emit_qtile(0, 1)
            vts[1] = emit_vt(1, ("act", "act"))
            emit_w(5, "gps")
            emit_qtile(0, 2)
            emit_hnt(2)
            emit_qtile(0, 3)
            emit_qtile(0, 4)
            emit_qtile(0, 5)
            emit_qtile(0, 6)
            emit_qtile(0, 7)

            for b in range(1, NB):
                for qt in range(NQT):
                    emit_qtile(b, qt)
                    if qt == 1 and b + 1 < NB:
                        vts[b + 1] = emit_vt(b + 1, ("act", "act"))
                    if qt == 3 and b + 2 < NB:
                        emit_hnt(b + 2)
            flush_pending()
    nc.compile()
    return nc


def _get_nc():
    if "nc" not in _CACHE:
        _CACHE["nc"] = _build()
    return _CACHE["nc"]


def _swap_halves(x):
    # swap the two key-column halves (involution)
    return np.concatenate([x[..., NQ:], x[..., :NQ]], axis=-1)


def make_in_maps(Hn, A_stat, M_mask, Wq, Wk):
    Hn = np.ascontiguousarray(np.asarray(Hn, dtype=np.float32))
    A_stat = np.asarray(A_stat, dtype=np.float32)
    M_mask = np.asarray(M_mask)
    Wq = np.asarray(Wq, dtype=np.float32)
    Wk = np.asarray(Wk, dtype=np.float32)
    assert Hn.shape == (B_FULL, N, D)

    # multiplicative bias: w = mask * a/(1-a), a = clip(A, eps, 1-eps)
    # additive form (ADD_QT slots): btab = ln(w) = logit(a); masked -> -69
    a = np.clip(A_stat, EPS, 1.0 - EPS)
    mask = M_mask != 0
    w_full = np.where(mask, a / (1.0 - a), 0.0).astype(np.float16)
    btab_full = np.where(
        mask, np.log(a) - np.log1p(-a), np.float32(-69.0)
    ).astype(np.float16)

    # G = (Wq^T Wk)/sqrt(D), packed [p, i, e] = G[i*128+p, e]
    G = (Wq.T @ Wk) * SCALE
    g_packed = np.ascontiguousarray(
        G.reshape(2, 128, D).transpose(1, 0, 2).astype(np.float16)
    )

    # [16, 256, 2048] transposed-node layout, fp16
    hnt_full = np.ascontiguousarray(Hn.astype(np.float16).transpose(0, 2, 1))

    in_maps = []
    for c in range(8):
        bg, qg = c // NQG, c % NQG
        bsl = slice(bg * NB, (bg + 1) * NB)
        qsl = slice(qg * NQ, (qg + 1) * NQ)
        hnt_c = hnt_full[bsl]
        w_c = w_full[qsl].reshape(NQT, 128, N).copy()
        b_c = btab_full[qsl].reshape(NQT, 128, N)
        for t in ADD_QT:
            w_c[t] = b_c[t]
        if qg == 1:
            hnt_c = _swap_halves(hnt_c)
            w_c = _swap_halves(w_c)
        in_maps.append({
            "hnt": np.ascontiguousarray(hnt_c),
            "w": np.ascontiguousarray(w_c),
            "g": g_packed,
            "idb": np.eye(128, dtype=np.float16),
        })
    return in_maps


def assemble(results):
    out = np.empty((B_FULL, N, N), dtype=np.float32)
    for c in range(8):
        bg, qg = c // NQG, c % NQG
        o = np.asarray(results[c]["o"])
        if qg == 1:
            o = _swap_halves(o)
        out[bg * NB:(bg + 1) * NB, qg * NQ:(qg + 1) * NQ, :] = (
            o.astype(np.float32)
        )
    return out


def kernel(Hn, A_stat, M_mask, Wq, Wk):
    in_maps = make_in_maps(Hn, A_stat, M_mask, Wq, Wk)
    nc = _get_nc()
    res = bass_utils.run_bass_kernel_spmd(nc, in_maps, core_ids=list(range(8)))
    return assemble(res.results)


if __name__ == "__main__":
    rng = np.random.default_rng(0)
    inputs = {
        "Hn": rng.standard_normal((B_FULL, N, D), dtype=np.float32),
        "A_stat": rng.random((N, N), dtype=np.float32),
        "M_mask": rng.integers(0, 2, size=(N, N), dtype=np.int32),
        "Wq": rng.standard_normal((D, D), dtype=np.float32) / 16,
        "Wk": rng.standard_normal((D, D), dtype=np.float32) / 16,
    }
    out = kernel(**inputs)
    print(out.shape, out.dtype, out.sum())
